# revision 18
# baseline (speedup 1.0000x reference)
"""Multi-head causal attention (B=4, T=2048, D=1024, 16 heads) on 8 TRN2 cores.

Sharding: core c -> batch b = c//2, head-group g = c%2 (8 of 16 heads).
Each core computes its batch's QKV for its heads, flash-style causal
attention with scores kept transposed (S^T[k, q]) so softmax sums come
free via a ones-column appended to V, then a partial output projection
y_part = attn_local @ W_proj[rows]. Host sums the two head-group partials
per batch.

Single fused pipeline per q-chunk (QKV for the chunk -> attention ->
projection) with all tile pools live at once, so the scheduler overlaps
next-chunk QKV matmuls into the ACT-paced attention stretches. x arrives
pre-transposed from the host (no device xbar transposes); weights load
per-c-tile across four queues so the first q-projection starts as soon
as ~384KB has landed instead of after the full 3.7MB weight set. y
stores at [128,512] granularity straight off each projection half.
Softmax normalization uses reciprocal_approx_fast plus a DRAM-bounce
row broadcast (ones-matmul for the last chunk). Projection drips are
deferred by one extra chunk (proj(0)->chunk2, proj(1,2)->chunk3) so the
final ACT-paced attention stretch has PE fill-in work.

Matmul operands are fp16; accumulation stays fp32 in PSUM.
"""

import math
from collections import deque
from contextlib import ExitStack

import numpy as np

import concourse.bacc as bacc
import concourse.bass as bass
import concourse.mybir as mybir
import concourse.tile as tile
from concourse.bass_utils import run_bass_kernel_spmd

AF = mybir.ActivationFunctionType
F32 = mybir.dt.float32
F16 = mybir.dt.float16

B_FULL = 4
T_FULL = 2048
D_FULL = 1024
NH_FULL = 16
HD = 64


def build_program(T, D, HL, n_pat, blocks):
    """Build the per-core SPMD program.

    T: sequence length, D: model dim, HL: local heads, n_pat: number of
    distinct mixed-mask pattern tiles, blocks: per q-chunk list of
    (k_tile_index, pattern_index_or_None) for active score blocks.
    """
    CL = HL * HD            # local channels (q, k, or v width)
    NDT = D // 128          # d-tiles (contraction tiles for qkv matmuls)
    NTT = T // 128          # t-tiles
    QCW = min(512, T)       # q-chunk width
    NQC = T // QCW
    TPC = QCW // 128        # t-tiles per q-chunk
    NCT = CL // 128         # c-tiles for q/k/attn storage
    PCH = min(512, D)       # proj output chunk
    NPCH = D // PCH
    scale = 1.0 / math.sqrt(HD)

    nc = bacc.Bacc("TRN2", target_bir_lowering=False, debug=False)
    # host-pre-tiled partition-major layouts so every load moves contiguous
    # multi-KB lines per partition (small-line APs run at ~1/4 DMA rate).
    # wq/wk are c-tile-major so the first projection's 256KB slab can land
    # ahead of the rest of the weight set.
    xh = nc.dram_tensor("xh", [NQC, 128, NDT, QCW], F16, kind="ExternalInput").ap()
    wqh = nc.dram_tensor("wqh", [NCT, 128, NDT, 128], F16, kind="ExternalInput").ap()
    wkh = nc.dram_tensor("wkh", [NCT, 128, NDT, 128], F16, kind="ExternalInput").ap()
    wvh = nc.dram_tensor("wvh", [128, NDT, CL], F16, kind="ExternalInput").ap()
    wph = nc.dram_tensor("wph", [128, NCT, D], F16, kind="ExternalInput").ap()
    # host-pre-broadcast bias pack: [bqs(NCT) | bks(NCT) | bvb(CL) | bpb(D)]
    BW = 2 * NCT + CL + D
    bh = nc.dram_tensor("bh", [128, BW], F32, kind="ExternalInput").ap()
    mp = nc.dram_tensor("mp", [max(n_pat, 1), 128, QCW], F16, kind="ExternalInput").ap()
    y = nc.dram_tensor("y", [T, D], F32, kind="ExternalOutput").ap()

    with tile.TileContext(nc) as tc, nc.allow_low_precision(
        reason="fp16 operands with fp32 PSUM accumulation; approx reciprocal"
    ):
        with ExitStack() as octx:
            persist = octx.enter_context(tc.tile_pool(name="persist", bufs=1))
            # K^T per c-tile, full sequence (grows as chunks complete)
            kT = [persist.tile([128, T], F16, name=f"kT{i}", tag=f"kT{i}") for i in range(NCT)]
            # small constants first: the HAM-warmup matmuls gate on onesW,
            # so it must clear the gpsimd queue quickly
            onesW = persist.tile([1, 128], F16, name="onesW", tag="onesW")
            nc.gpsimd.memset(onesW, 1.0)
            # -ln(16) bias column for the scaled exp
            nbias = persist.tile([128, 1], F32, name="nbias", tag="nbias")
            nc.gpsimd.memset(nbias, -math.log(16.0))
            # V in per-(t-tile, head) 128-wide slots: cols [0:HD) data, [HD]
            # ones (for the free softmax row-sum), rest junk. The junk lanes
            # only feed matmul out-partitions that are never read, so they
            # stay uninitialized on purpose (no startup memset wave).
            vS = persist.tile([128, NTT, HL, 128], F16, name="vS", tag="vS")

            # startup DMA plan, first-needed-first across four queues:
            #   sync:   bias qk slice, x chunk0 (d-tiles 0:2 then 2:8)
            #   scalar: wq per c-tile (first q matmul needs only ct 0)
            #   gpsimd: wk per c-tile (behind the small memsets), bpb, wps
            #   vector: wv, bvb
            bpack = persist.tile([128, BW], F32, name="bpack", tag="bpack")
            bqs = bpack[:, 0:NCT]
            bks = bpack[:, NCT:2 * NCT]
            bvb = bpack[:, 2 * NCT:2 * NCT + CL]
            bpb = bpack[:, 2 * NCT + CL:BW]
            nc.sync.dma_start(out=bpack[:, 0:2 * NCT], in_=bh[:, 0:2 * NCT])
            wqt = persist.tile([128, NCT, NDT, 128], F16, name="wqt", tag="wqt")
            wkt = persist.tile([128, NCT, NDT, 128], F16, name="wkt", tag="wkt")
            for mc in range(NCT):
                nc.scalar.dma_start(out=wqt[:, mc], in_=wqh[mc])
            wvt = persist.tile([128, NDT, CL], F16, name="wvt", tag="wvt")
            nc.scalar.dma_start(out=wvt, in_=wvh)
            nc.scalar.dma_start(
                out=bpack[:, 2 * NCT:2 * NCT + CL], in_=bh[:, 2 * NCT:2 * NCT + CL]
            )
            mts = [persist.tile([128, QCW], F16, name=f"mt{i}", tag=f"mt{i}") for i in range(n_pat)]
            for i in range(n_pat):
                nc.scalar.dma_start(out=mts[i], in_=mp[i])

            # preload the exp table set during the initial DMA wait
            warmup = persist.tile([1, 16], F32, name="warm", tag="warm")
            nc.gpsimd.memset(warmup, 0.0)
            nc.scalar.activation(warmup, warmup, AF.Exp, scale=1.0)

            pools = ExitStack()
            with pools:
                xtp = pools.enter_context(tc.tile_pool(name="xtp", bufs=2))
                qtp = pools.enter_context(tc.tile_pool(name="qtp", bufs=2))
                attnp = pools.enter_context(tc.tile_pool(name="attnp", bufs=4))
                ptl = pools.enter_context(tc.tile_pool(name="ptl", bufs=3))
                rip = pools.enter_context(tc.tile_pool(name="rip", bufs=4))
                drp = pools.enter_context(tc.tile_pool(name="drp", bufs=4, space="DRAM"))
                ysb = pools.enter_context(tc.tile_pool(name="ysb", bufs=3))
                pss = pools.enter_context(tc.tile_pool(name="pss", bufs=2, space="PSUM"))
                psav = pools.enter_context(tc.tile_pool(name="psav", bufs=2, space="PSUM"))
                pwork = pools.enter_context(tc.tile_pool(name="pwork", bufs=2, space="PSUM"))

                # k weight DMAs head the gpsimd queue (issue-only, ~640ns
                # each) so the k projections aren't gated on memset waves.
                for mc in range(NCT):
                    nc.gpsimd.dma_start(out=wkt[:, mc], in_=wkh[mc])
                # qT constant-zero halves for the ping-pong parity scheme:
                # parity e stores q in rows [0:HD) (rows [HD:128) stay 0),
                # parity o in rows [HD:128). The zero rows kill the other
                # head's channels in the 128-deep scores contraction. Only
                # chunk-0's buffer heads the DVE queue (must beat the first
                # bias-add there); everything not needed before ~30us goes
                # behind the gpsimd DMA issues or after the qkv0 emission.
                qbufs = [qtp.tile([128, 2, NCT, QCW], F16, name="qT", tag="qT")
                         for _ in range(2)]
                nc.vector.memset(qbufs[0][HD:128, 0], 0.0)
                nc.vector.memset(qbufs[0][0:HD, 1], 0.0)
                nc.gpsimd.memset(vS[:, :, :, HD:HD + 1], 1.0)
                # initialize the pT ring (trimmed exp leaves leading columns
                # stale on later passes; affine_select re-fills them)
                wu_pt = [ptl.tile([128, 2 * QCW], F16, name="pT", tag="pT") for _ in range(3)]
                for t in wu_pt:
                    nc.gpsimd.memset(t, 0.0)
                wps = persist.tile([128, NCT, D], F16, name="wps", tag="wps")
                nc.gpsimd.dma_start(
                    out=bpack[:, 2 * NCT + CL:BW], in_=bh[:, 2 * NCT + CL:BW]
                )
                nc.gpsimd.dma_start(out=wps, in_=wph)

                # HAM warmup: keep the PE busy during the initial DMA wait so
                # the first real matmuls run at full clock. The first writes
                # fully initialize both pss pair buffers so the diagonal
                # trims can later leave stale (but initialized) PSUM columns.
                wu_ps = [pss.tile([128, 2 * QCW], F32, name="pS", tag="pS") for _ in range(2)]
                for i in range(2 * (2 * QCW // 128)):
                    dst = wu_ps[i % 2][:, (i // 2) * 128:(i // 2) * 128 + 128]
                    nc.tensor.matmul(dst, lhsT=onesW, rhs=onesW[:, 0:128],
                                     start=True, stop=True)
                wdum = pwork.tile([128, QCW], F32, name="wdum", tag="pw")
                for i in range(4):
                    nc.tensor.matmul(wdum[:, 0:128], lhsT=onesW, rhs=onesW[:, 0:128],
                                     start=(i == 0), stop=(i == 3))

                def qkv_chunk(ntc):
                    """QKV projections for q-chunk ntc: fills vS t-tiles,
                    kT columns, and the qT ping-pong buffer for this chunk."""
                    xTc = xtp.tile([128, NDT, QCW], F16, name="xTc", tag="xTc")
                    if ntc == 0:
                        # per-d-tile pieces so the first q matmuls chase the
                        # DMA arrivals instead of waiting for the full 1MB
                        for dd in range(NDT):
                            nc.sync.dma_start(out=xTc[:, dd:dd + 1], in_=xh[0][:, dd:dd + 1])
                    else:
                        nc.sync.dma_start(out=xTc, in_=xh[ntc])
                    qt = qbufs[ntc % 2]

                    def qk_group(isq, mc):
                        wt = wqt if isq else wkt
                        pb = pwork.tile([128, QCW], F32, name="pb", tag="pw")
                        for dd in range(NDT):
                            nc.tensor.matmul(
                                pb,
                                lhsT=wt[:, mc, dd, :],
                                rhs=xTc[:, dd, :],
                                start=(dd == 0),
                                stop=(dd == NDT - 1),
                            )
                        if isq:
                            nc.vector.tensor_scalar_add(
                                qt[0:HD, 0, mc, :], pb[0:HD, :], bqs[0:HD, mc:mc + 1]
                            )
                            nc.vector.tensor_scalar_add(
                                qt[HD:128, 1, mc, :], pb[HD:128, :], bqs[HD:128, mc:mc + 1]
                            )
                        else:
                            nc.vector.tensor_scalar_add(
                                kT[mc][:, ntc * QCW:(ntc + 1) * QCW], pb, bks[:, mc:mc + 1]
                            )

                    def v_group(tv):
                        tt = ntc * TPC + tv
                        pv = pwork.tile([128, CL], F32, name="pv", tag="pw")
                        for dd in range(NDT):
                            nc.tensor.matmul(
                                pv,
                                lhsT=xTc[:, dd, tv * 128:(tv + 1) * 128],
                                rhs=wvt[:, dd, :],
                                start=(dd == 0),
                                stop=(dd == NDT - 1),
                            )
                        nc.vector.tensor_add(
                            vS[:, tt, :, 0:HD],
                            pv.rearrange("p (h d) -> p h d", h=HL),
                            bvb.rearrange("p (h d) -> p h d", h=HL),
                        )

                    # q/k first: their weights stream in c-tile order, so
                    # compute starts before the v/k weight sets finish
                    for mc in range(NCT):
                        qk_group(True, mc)
                        qk_group(False, mc)
                    for tv in range(TPC):
                        v_group(tv)

                qkv_chunk(0)
                nc.vector.memset(qbufs[1][HD:128, 0], 0.0)
                nc.vector.memset(qbufs[1][0:HD, 1], 0.0)

                def block_trim(qc, pat):
                    """Fully-masked leading q-columns of a diagonal block
                    (skipped in matmuls and exp; affine_select re-zeroes
                    whatever stale values sit there)."""
                    if pat is not None and pat[0] == "tri" and pat[1] < 0:
                        return -pat[1]
                    return 0

                # projection drip queue: one (qc, attnT, tv, nch) unit per
                # [128, PCH] output block. All of chunks 0-2's units drain
                # inside chunk 3's head loops (the only ACT-paced stretch
                # without a next-chunk QKV reservoir), remainder at the tail.
                drips = deque()

                def proj_unit(store_q=None):
                    """Emit one (t-tile, out-chunk) projection unit of a
                    finished q-chunk; interleaved into later head loops so
                    its matmuls fill ACT-paced PE gaps. Stores its half-row
                    immediately to keep the kernel tail short."""
                    if not drips:
                        return
                    qc, attnT, tv, nch = drips.popleft()
                    py = pwork.tile([128, PCH], F32, name="py", tag="pw")
                    for cc in range(NCT):
                        nc.tensor.matmul(
                            py,
                            lhsT=attnT[cc][:, tv * 128:(tv + 1) * 128],
                            rhs=wps[:, cc, nch * PCH:(nch + 1) * PCH],
                            start=(cc == 0),
                            stop=(cc == NCT - 1),
                        )
                    yt = ysb.tile([128, PCH], F32, name="yt", tag="yt")
                    nc.vector.tensor_add(yt, py, bpb[:, nch * PCH:(nch + 1) * PCH])
                    tt = qc * TPC + tv
                    (store_q or nc.sync).dma_start(
                        out=y[tt * 128:(tt + 1) * 128, nch * PCH:(nch + 1) * PCH],
                        in_=yt,
                    )

                assert NQC == 4, "drip schedule is tuned for 4 q-chunks"
                drips_per_head = {0: 0, 1: 0, 2: 0, 3: 3}

                for qc in range(NQC):
                    row = blocks[qc]
                    assert row, f"q-chunk {qc} has no active k-tiles"
                    qt = qbufs[qc % 2]
                    attnT = [attnp.tile([128, QCW], F16, name=f"attnT{i}", tag=f"attnT{i}")
                             for i in range(NCT)]
                    npr = (len(row) + 1) // 2
                    dp = drips_per_head[qc]
                    # late-biased drip positions: the exp-paced PE deficit
                    # shows up at the head's trailing pairs / head boundary
                    drip_at = {2 * (npr - 1 - 2 * j) for j in range(dp)}
                    for h in range(HL):
                        mc, par = h // 2, h % 2
                        pav = psav.tile([128, QCW], F32, name="pav", tag="pav")
                        for pi in range(0, len(row), 2):
                            if pi in drip_at:
                                proj_unit()
                            pair = row[pi:pi + 2]
                            w = len(pair) * QCW
                            pS = pss.tile([128, 2 * QCW], F32, name="pS", tag="pS")
                            for sj, (ki, pat) in enumerate(pair):
                                trim = block_trim(qc, pat)
                                nc.tensor.matmul(
                                    pS[:, sj * QCW + trim:(sj + 1) * QCW],
                                    lhsT=kT[mc][:, ki * 128:(ki + 1) * 128],
                                    rhs=qt[:, par, mc, trim:QCW],
                                    start=True,
                                    stop=True,
                                )
                            # exp(scale*s - ln16): the 1/16 keeps l within
                            # fp16 range for the broadcast matmul; it cancels
                            # in attn = (pav/16) * (16/l).
                            t0 = block_trim(qc, pair[0][1])
                            pT = ptl.tile([128, 2 * QCW], F16, name="pT", tag="pT")
                            nc.scalar.activation(pT[:, t0:w], pS[:, t0:w], AF.Exp,
                                                 scale=scale, bias=nbias)
                            for sj, (ki, pat) in enumerate(pair):
                                sl = pT[:, sj * QCW:(sj + 1) * QCW]
                                if pat is not None:
                                    kind, arg = pat
                                    if kind == "tri":
                                        # keep where (q - k) >= 0, else 0
                                        nc.gpsimd.affine_select(
                                            out=sl,
                                            in_=sl,
                                            pattern=[[1, QCW]],
                                            base=arg,
                                            channel_multiplier=-1,
                                            compare_op=mybir.AluOpType.is_ge,
                                            fill=0.0,
                                        )
                                    else:
                                        nc.gpsimd.tensor_mul(sl, sl, mts[arg])
                                nc.tensor.matmul(
                                    pav,
                                    lhsT=vS[:, ki, h],
                                    rhs=sl,
                                    start=(pi == 0 and sj == 0),
                                    stop=(pi + sj == len(row) - 1),
                                )
                        # normalize: 1/l on DVE (SBUF-fed fast reciprocal),
                        # broadcast across HD partitions via a DRAM bounce on
                        # the sync queue, then scale. (A gpsimd
                        # partition_broadcast would be one op, but it thrashes
                        # the DSP library against affine_select — 5us reload
                        # stalls.) For the last chunk the bounce latency is
                        # the kernel tail, so broadcast with a PE ones-matmul
                        # + DVE copy instead.
                        lsb = rip.tile([1, QCW], F32, name="lsb", tag="lsb")
                        nc.vector.tensor_scalar_mul(lsb, pav[HD:HD + 1, :], 1.0)
                        rinv = rip.tile([1, QCW], F32, name="rinv", tag="rinv")
                        nc.vector.reciprocal_approx_fast(out=rinv, in_=lsb)
                        if qc + 1 < NQC:
                            scr = drp.tile([QCW], F32, name="scr", tag="scr")
                            nc.sync.dma_start(out=scr, in_=rinv)
                            rbs = rip.tile([HD, QCW], F32, name="rbs", tag="rbs")
                            nc.sync.dma_start(
                                out=rbs,
                                in_=bass.AP(tensor=scr.tensor, offset=scr.offset,
                                            ap=[[0, HD]] + list(scr.ap)),
                            )
                        else:
                            rinv16 = rip.tile([1, QCW], F16, name="rinv16", tag="rinv16")
                            nc.vector.tensor_scalar_mul(rinv16, rinv, 1.0)
                            bcR = pwork.tile([128, QCW], F32, name="bcR", tag="pw")
                            nc.tensor.matmul(bcR, lhsT=onesW, rhs=rinv16,
                                             start=True, stop=True)
                            rbs = rip.tile([HD, QCW], F32, name="rbs", tag="rbs")
                            nc.vector.tensor_scalar_mul(rbs, bcR[0:HD, :], 1.0)
                        nc.vector.tensor_mul(
                            attnT[mc][par * HD:(par + 1) * HD, :],
                            pav[0:HD, :],
                            rbs,
                        )
                    if qc + 1 < NQC:
                        qkv_chunk(qc + 1)
                    for tv in range(TPC):
                        for nch in range(NPCH):
                            drips.append((qc, attnT, tv, nch))
                # tail drain: rotate stores across all three DMA queues so
                # the final writes don't serialize behind one another
                tail_qs = [nc.sync, nc.gpsimd, nc.scalar]
                ti = 0
                while drips:
                    proj_unit(store_q=tail_qs[ti % 3])
                    ti += 1
    nc.compile()
    return nc


def classify_mask(mask_bool, T):
    """Classify S^T blocks [k-tile 128, q-chunk 512] as skip / full / mixed.

    mask_bool: [T, T] bool, mask_bool[q, k] = attend(q -> k).
    Returns (blocks, patterns): blocks[qc] = list of (ki, pat_idx|None),
    patterns = np.ndarray [n_pat, 128, QCW] float32.
    """
    QCW = min(512, T)
    NQC = T // QCW
    NKT = T // 128
    maskT = mask_bool.T  # [k, q]
    patterns = []
    pat_index = {}
    blocks = []
    for qc in range(NQC):
        row = []
        for ki in range(NKT):
            blk = maskT[ki * 128:(ki + 1) * 128, qc * QCW:(qc + 1) * QCW]
            if not blk.any():
                continue
            if blk.all():
                row.append((ki, None))
                continue
            # tril-offset block? keep iff k <= q, i.e. p <= base + f
            base = qc * QCW - ki * 128
            p = np.arange(128)[:, None]
            f = np.arange(QCW)[None, :]
            if np.array_equal(blk, p <= base + f):
                row.append((ki, ("tri", base)))
                continue
            key = blk.tobytes()
            if key not in pat_index:
                pat_index[key] = len(patterns)
                patterns.append(blk.astype(np.float32))
            row.append((ki, ("pat", pat_index[key])))
        blocks.append(row)
    n_pat = len(patterns)
    if patterns:
        pats = np.stack(patterns)
    else:
        pats = np.zeros((1, 128, QCW), np.float32)
    return blocks, pats, n_pat


_prog_cache = {}


def _get_program(T, D, HL, mask_bool):
    key = (T, D, HL, mask_bool.tobytes())
    if key not in _prog_cache:
        blocks, pats, n_pat = classify_mask(mask_bool, T)
        nc = build_program(T, D, HL, n_pat, blocks)
        _prog_cache[key] = (nc, blocks, pats)
    return _prog_cache[key]


def kernel(x, W_qkv, b_qkv, W_proj, b_proj, mask):
    out, _ = run_attention(x, W_qkv, b_qkv, W_proj, b_proj, mask)
    return out


def run_attention(x, W_qkv, b_qkv, W_proj, b_proj, mask, trace=False):
    x = np.ascontiguousarray(np.asarray(x, dtype=np.float32))
    W_qkv = np.asarray(W_qkv, dtype=np.float32)
    b_qkv = np.asarray(b_qkv, dtype=np.float32)
    W_proj = np.asarray(W_proj, dtype=np.float32)
    b_proj = np.asarray(b_proj, dtype=np.float32)
    Bc, T, D = x.shape
    NH = NH_FULL
    HL = NH // 2  # heads per core (two head-groups)
    CL = HL * HD

    mask_bool = np.asarray(mask)[0, 0] != 0

    nc, blocks, pats = _get_program(T, D, HL, mask_bool)

    NDT = D // 128
    NCT = CL // 128
    QCW = min(512, T)
    NQC = T // QCW

    def tile_w(w, inner):
        # [D_rows, W_cols] -> [128(p), D_rows//128(n), *inner] partition-major
        return np.ascontiguousarray(
            w.reshape(w.shape[0] // 128, 128, *inner).transpose(
                1, 0, *range(2, 2 + len(inner)))
        ).astype(np.float16)

    def tile_w_ct(w):
        # [D_rows, NCT*128] -> [NCT, 128(p), NDT, 128] c-tile-major
        return np.ascontiguousarray(
            w.reshape(NDT, 128, NCT, 128).transpose(2, 1, 0, 3)
        ).astype(np.float16)

    in_maps = []
    n_cores = 2 * Bc
    for c in range(n_cores):
        b, g = c // 2, c % 2
        sl = slice(g * CL, (g + 1) * CL)
        xT = x[b].T  # [D, T]
        xhp = np.ascontiguousarray(
            xT.reshape(NDT, 128, NQC, QCW).transpose(2, 1, 0, 3)
        ).astype(np.float16)
        bq = b_qkv[0 * D:1 * D][sl]
        bk = b_qkv[1 * D:2 * D][sl]
        bv = b_qkv[2 * D:3 * D][sl]
        bp = b_proj if g == 0 else np.zeros_like(b_proj)
        bh = np.concatenate([
            bq.reshape(NCT, 128).T,           # [128, NCT] per-partition bqs
            bk.reshape(NCT, 128).T,           # [128, NCT]
            np.broadcast_to(bv, (128, CL)),   # [128, CL]
            np.broadcast_to(bp, (128, D)),    # [128, D]
        ], axis=1).astype(np.float32)
        in_maps.append({
            "xh": xhp,
            "wqh": tile_w_ct(W_qkv[:, 0 * D:1 * D][:, sl]),
            "wkh": tile_w_ct(W_qkv[:, 1 * D:2 * D][:, sl]),
            "wvh": tile_w(W_qkv[:, 2 * D:3 * D][:, sl], (CL,)),
            "wph": tile_w(W_proj[sl, :], (D,)),
            "bh": np.ascontiguousarray(bh),
            "mp": pats.astype(np.float16),
        })

    res = run_bass_kernel_spmd(nc, in_maps, list(range(n_cores)), trace=trace)
    out = np.empty((Bc, T, D), np.float32)
    for b in range(Bc):
        out[b] = res.results[2 * b]["y"] + res.results[2 * b + 1]["y"]
    return out, res


# revision 23
# speedup vs baseline: 1.0577x; 1.0577x over previous
"""Multi-head causal attention (B=4, T=2048, D=1024, 16 heads) on 8 TRN2 cores.

Sharding: core c -> batch b = c//2, head-group g = c%2 (8 of 16 heads).
Each core computes its batch's QKV for its heads, flash-style causal
attention with scores kept transposed (S^T[k, q]) so softmax sums come
free via a ones-column appended to V, then a partial output projection
y_part = attn_local @ W_proj[rows]. Host sums the two head-group partials
per batch.

Single fused pipeline per q-chunk (QKV for the chunk -> attention ->
projection) with all tile pools live at once, so the scheduler overlaps
next-chunk QKV matmuls into the ACT-paced attention stretches. x arrives
pre-transposed from the host (no device xbar transposes); weights load
per-c-tile across four queues so the first q-projection starts as soon
as ~384KB has landed instead of after the full 3.7MB weight set. y
stores at [128,512] granularity straight off each projection half.
Softmax normalization uses reciprocal_approx_fast plus a DRAM-bounce
row broadcast (ones-matmul for the last chunk). Projection drips are
deferred by one extra chunk (proj(0)->chunk2, proj(1,2)->chunk3) so the
final ACT-paced attention stretch has PE fill-in work.

Matmul operands are fp16; accumulation stays fp32 in PSUM.
"""

import math
from collections import deque
from contextlib import ExitStack

import numpy as np

import concourse.bacc as bacc
import concourse.bass as bass
import concourse.mybir as mybir
import concourse.tile as tile
from concourse.bass_utils import run_bass_kernel_spmd

AF = mybir.ActivationFunctionType
F32 = mybir.dt.float32
F16 = mybir.dt.float16

B_FULL = 4
T_FULL = 2048
D_FULL = 1024
NH_FULL = 16
HD = 64


def build_program(T, D, HL, n_pat, blocks):
    """Build the per-core SPMD program.

    T: sequence length, D: model dim, HL: local heads, n_pat: number of
    distinct mixed-mask pattern tiles, blocks: per q-chunk list of
    (k_tile_index, pattern_index_or_None) for active score blocks.
    """
    CL = HL * HD            # local channels (q, k, or v width)
    NDT = D // 128          # d-tiles (contraction tiles for qkv matmuls)
    NTT = T // 128          # t-tiles
    QCW = min(512, T)       # q-chunk width
    NQC = T // QCW
    TPC = QCW // 128        # t-tiles per q-chunk
    NCT = CL // 128         # c-tiles for q/k/attn storage
    PCH = min(512, D)       # proj output chunk
    NPCH = D // PCH
    scale = 1.0 / math.sqrt(HD)

    nc = bacc.Bacc("TRN2", target_bir_lowering=False, debug=False)
    # host-pre-tiled partition-major layouts so every load moves contiguous
    # multi-KB lines per partition (small-line APs run at ~1/4 DMA rate).
    # wq/wk are c-tile-major so the first projection's 256KB slab can land
    # ahead of the rest of the weight set.
    xh = nc.dram_tensor("xh", [NQC, 128, NDT, QCW], F16, kind="ExternalInput").ap()
    wqh = nc.dram_tensor("wqh", [NCT, 128, NDT, 128], F16, kind="ExternalInput").ap()
    wkh = nc.dram_tensor("wkh", [NCT, 128, NDT, 128], F16, kind="ExternalInput").ap()
    wvh = nc.dram_tensor("wvh", [128, NDT, CL], F16, kind="ExternalInput").ap()
    wph = nc.dram_tensor("wph", [128, NCT, D], F16, kind="ExternalInput").ap()
    # host-pre-broadcast bias pack: [bqs(NCT) | bks(NCT) | bvb(CL) | bpb(D)]
    BW = 2 * NCT + CL + D
    bh = nc.dram_tensor("bh", [128, BW], F32, kind="ExternalInput").ap()
    mp = nc.dram_tensor("mp", [max(n_pat, 1), 128, QCW], F16, kind="ExternalInput").ap()
    y = nc.dram_tensor("y", [T, D], F32, kind="ExternalOutput").ap()

    with tile.TileContext(nc) as tc, nc.allow_low_precision(
        reason="fp16 operands with fp32 PSUM accumulation; approx reciprocal"
    ):
        with ExitStack() as octx:
            persist = octx.enter_context(tc.tile_pool(name="persist", bufs=1))
            # K^T per c-tile, full sequence (grows as chunks complete)
            kT = [persist.tile([128, T], F16, name=f"kT{i}", tag=f"kT{i}") for i in range(NCT)]
            # small constants first: the HAM-warmup matmuls gate on onesW,
            # so it must clear the gpsimd queue quickly
            onesW = persist.tile([1, 128], F16, name="onesW", tag="onesW")
            nc.gpsimd.memset(onesW, 1.0)
            # -ln(16) bias column for the scaled exp
            nbias = persist.tile([128, 1], F32, name="nbias", tag="nbias")
            nc.gpsimd.memset(nbias, -math.log(16.0))
            # V in per-(t-tile, head) 128-wide slots: cols [0:HD) data, [HD]
            # ones (for the free softmax row-sum), rest junk. The junk lanes
            # only feed matmul out-partitions that are never read, so they
            # stay uninitialized on purpose (no startup memset wave).
            vS = persist.tile([128, NTT, HL, 128], F16, name="vS", tag="vS")

            # startup DMA plan, first-needed-first across four queues:
            #   sync:   bias qk slice, x chunk0 (d-tiles 0:2 then 2:8)
            #   scalar: wq per c-tile (first q matmul needs only ct 0)
            #   gpsimd: wk per c-tile (behind the small memsets), bpb, wps
            #   vector: wv, bvb
            bpack = persist.tile([128, BW], F32, name="bpack", tag="bpack")
            bqs = bpack[:, 0:NCT]
            bks = bpack[:, NCT:2 * NCT]
            bvb = bpack[:, 2 * NCT:2 * NCT + CL]
            bpb = bpack[:, 2 * NCT + CL:BW]
            nc.sync.dma_start(out=bpack[:, 0:2 * NCT], in_=bh[:, 0:2 * NCT])
            wqt = persist.tile([128, NCT, NDT, 128], F16, name="wqt", tag="wqt")
            wkt = persist.tile([128, NCT, NDT, 128], F16, name="wkt", tag="wkt")
            for mc in range(NCT):
                nc.scalar.dma_start(out=wqt[:, mc], in_=wqh[mc])
            wvt = persist.tile([128, NDT, CL], F16, name="wvt", tag="wvt")
            nc.scalar.dma_start(out=wvt, in_=wvh)
            nc.scalar.dma_start(
                out=bpack[:, 2 * NCT:2 * NCT + CL], in_=bh[:, 2 * NCT:2 * NCT + CL]
            )
            mts = [persist.tile([128, QCW], F16, name=f"mt{i}", tag=f"mt{i}") for i in range(n_pat)]
            for i in range(n_pat):
                nc.scalar.dma_start(out=mts[i], in_=mp[i])

            # preload the exp table set during the initial DMA wait
            warmup = persist.tile([1, 16], F32, name="warm", tag="warm")
            nc.gpsimd.memset(warmup, 0.0)
            nc.scalar.activation(warmup, warmup, AF.Exp, scale=1.0)

            pools = ExitStack()
            with pools:
                xtp = pools.enter_context(tc.tile_pool(name="xtp", bufs=2))
                qtp = pools.enter_context(tc.tile_pool(name="qtp", bufs=2))
                attnp = pools.enter_context(tc.tile_pool(name="attnp", bufs=4))
                ptl = pools.enter_context(tc.tile_pool(name="ptl", bufs=3))
                rip = pools.enter_context(tc.tile_pool(name="rip", bufs=4))
                drp = pools.enter_context(tc.tile_pool(name="drp", bufs=4, space="DRAM"))
                ysb = pools.enter_context(tc.tile_pool(name="ysb", bufs=3))
                pss = pools.enter_context(tc.tile_pool(name="pss", bufs=2, space="PSUM"))
                psav = pools.enter_context(tc.tile_pool(name="psav", bufs=2, space="PSUM"))
                pwork = pools.enter_context(tc.tile_pool(name="pwork", bufs=2, space="PSUM"))

                # k weight DMAs head the gpsimd queue (issue-only, ~640ns
                # each) so the k projections aren't gated on memset waves.
                for mc in range(NCT):
                    nc.gpsimd.dma_start(out=wkt[:, mc], in_=wkh[mc])
                # qT constant-zero halves for the ping-pong parity scheme:
                # parity e stores q in rows [0:HD) (rows [HD:128) stay 0),
                # parity o in rows [HD:128). The zero rows kill the other
                # head's channels in the 128-deep scores contraction. Only
                # chunk-0's buffer heads the DVE queue (must beat the first
                # bias-add there); everything not needed before ~30us goes
                # behind the gpsimd DMA issues or after the qkv0 emission.
                qbufs = [qtp.tile([128, 2, NCT, QCW], F16, name="qT", tag="qT")
                         for _ in range(2)]
                nc.vector.memset(qbufs[0][HD:128, 0], 0.0)
                nc.vector.memset(qbufs[0][0:HD, 1], 0.0)
                nc.gpsimd.memset(vS[:, :, :, HD:HD + 1], 1.0)
                # initialize the pT ring (trimmed exp leaves leading columns
                # stale on later passes; affine_select re-fills them)
                wu_pt = [ptl.tile([128, 2 * QCW], F16, name="pT", tag="pT") for _ in range(3)]
                for t in wu_pt:
                    nc.gpsimd.memset(t, 0.0)
                wps = persist.tile([128, NCT, D], F16, name="wps", tag="wps")
                nc.gpsimd.dma_start(
                    out=bpack[:, 2 * NCT + CL:BW], in_=bh[:, 2 * NCT + CL:BW]
                )
                nc.gpsimd.dma_start(out=wps, in_=wph)

                # HAM warmup: keep the PE busy during the initial DMA wait so
                # the first real matmuls run at full clock. The first writes
                # fully initialize both pss pair buffers so the diagonal
                # trims can later leave stale (but initialized) PSUM columns.
                wu_ps = [pss.tile([128, 2 * QCW], F32, name="pS", tag="pS") for _ in range(2)]
                for i in range(2 * (2 * QCW // 128)):
                    dst = wu_ps[i % 2][:, (i // 2) * 128:(i // 2) * 128 + 128]
                    nc.tensor.matmul(dst, lhsT=onesW, rhs=onesW[:, 0:128],
                                     start=True, stop=True)
                wdum = pwork.tile([128, QCW], F32, name="wdum", tag="pw")
                for i in range(4):
                    nc.tensor.matmul(wdum[:, 0:128], lhsT=onesW, rhs=onesW[:, 0:128],
                                     start=(i == 0), stop=(i == 3))

                def qkv_chunk(ntc):
                    """QKV projections for q-chunk ntc: fills vS t-tiles,
                    kT columns, and the qT ping-pong buffer for this chunk."""
                    # one DMA per chunk: a single descriptor set spreads
                    # across all 16 rings, so 1MB lands in ~3us — faster
                    # than any split with its serial descriptor-gen cost
                    xTc = xtp.tile([128, NDT, QCW], F16, name="xTc", tag="xTc")
                    nc.sync.dma_start(out=xTc, in_=xh[ntc])
                    qt = qbufs[ntc % 2]

                    def qk_group(isq, mc):
                        wt = wqt if isq else wkt
                        pb = pwork.tile([128, QCW], F32, name="pb", tag="pw")
                        for dd in range(NDT):
                            nc.tensor.matmul(
                                pb,
                                lhsT=wt[:, mc, dd, :],
                                rhs=xTc[:, dd, :],
                                start=(dd == 0),
                                stop=(dd == NDT - 1),
                            )
                        if isq:
                            nc.vector.tensor_scalar_add(
                                qt[0:HD, 0, mc, :], pb[0:HD, :], bqs[0:HD, mc:mc + 1]
                            )
                            nc.vector.tensor_scalar_add(
                                qt[HD:128, 1, mc, :], pb[HD:128, :], bqs[HD:128, mc:mc + 1]
                            )
                        else:
                            nc.vector.tensor_scalar_add(
                                kT[mc][:, ntc * QCW:(ntc + 1) * QCW], pb, bks[:, mc:mc + 1]
                            )

                    def v_group(tv):
                        tt = ntc * TPC + tv
                        pv = pwork.tile([128, CL], F32, name="pv", tag="pw")
                        for dd in range(NDT):
                            nc.tensor.matmul(
                                pv,
                                lhsT=xTc[:, dd, tv * 128:(tv + 1) * 128],
                                rhs=wvt[:, dd, :],
                                start=(dd == 0),
                                stop=(dd == NDT - 1),
                            )
                        nc.vector.tensor_add(
                            vS[:, tt, :, 0:HD],
                            pv.rearrange("p (h d) -> p h d", h=HL),
                            bvb.rearrange("p (h d) -> p h d", h=HL),
                        )

                    # q/k first: their weights stream in c-tile order, so
                    # compute starts before the v/k weight sets finish
                    for mc in range(NCT):
                        qk_group(True, mc)
                        qk_group(False, mc)
                    for tv in range(TPC):
                        v_group(tv)

                qkv_chunk(0)
                nc.vector.memset(qbufs[1][HD:128, 0], 0.0)
                nc.vector.memset(qbufs[1][0:HD, 1], 0.0)

                def block_trim(qc, pat):
                    """Fully-masked leading q-columns of a diagonal block
                    (skipped in matmuls and exp; affine_select re-zeroes
                    whatever stale values sit there)."""
                    if pat is not None and pat[0] == "tri" and pat[1] < 0:
                        return -pat[1]
                    return 0

                # projection drip queue: one (qc, attnT, tv, nch) unit per
                # [128, PCH] output block. All of chunks 0-2's units drain
                # inside chunk 3's head loops (the only ACT-paced stretch
                # without a next-chunk QKV reservoir), remainder at the tail.
                drips = deque()

                def proj_unit(store_q=None):
                    """Emit one (t-tile, out-chunk) projection unit of a
                    finished q-chunk; interleaved into later head loops so
                    its matmuls fill ACT-paced PE gaps. Stores its half-row
                    immediately to keep the kernel tail short."""
                    if not drips:
                        return
                    qc, attnT, tv, nch = drips.popleft()
                    py = pwork.tile([128, PCH], F32, name="py", tag="pw")
                    for cc in range(NCT):
                        nc.tensor.matmul(
                            py,
                            lhsT=attnT[cc][:, tv * 128:(tv + 1) * 128],
                            rhs=wps[:, cc, nch * PCH:(nch + 1) * PCH],
                            start=(cc == 0),
                            stop=(cc == NCT - 1),
                        )
                    yt = ysb.tile([128, PCH], F32, name="yt", tag="yt")
                    nc.vector.tensor_add(yt, py, bpb[:, nch * PCH:(nch + 1) * PCH])
                    tt = qc * TPC + tv
                    (store_q or nc.sync).dma_start(
                        out=y[tt * 128:(tt + 1) * 128, nch * PCH:(nch + 1) * PCH],
                        in_=yt,
                    )

                assert NQC == 4, "drip schedule is tuned for 4 q-chunks"
                drips_per_head = {0: 0, 1: 0, 2: 0, 3: 3}

                for qc in range(NQC):
                    row = blocks[qc]
                    assert row, f"q-chunk {qc} has no active k-tiles"
                    qt = qbufs[qc % 2]
                    attnT = [attnp.tile([128, QCW], F16, name=f"attnT{i}", tag=f"attnT{i}")
                             for i in range(NCT)]
                    npr = (len(row) + 1) // 2
                    dp = drips_per_head[qc]
                    # late-biased drip positions: the exp-paced PE deficit
                    # shows up at the head's trailing pairs / head boundary
                    drip_at = {2 * (npr - 1 - 2 * j) for j in range(dp)}
                    for h in range(HL):
                        mc, par = h // 2, h % 2
                        pav = psav.tile([128, QCW], F32, name="pav", tag="pav")
                        for pi in range(0, len(row), 2):
                            if pi in drip_at:
                                proj_unit()
                            pair = row[pi:pi + 2]
                            w = len(pair) * QCW
                            pS = pss.tile([128, 2 * QCW], F32, name="pS", tag="pS")
                            for sj, (ki, pat) in enumerate(pair):
                                trim = block_trim(qc, pat)
                                nc.tensor.matmul(
                                    pS[:, sj * QCW + trim:(sj + 1) * QCW],
                                    lhsT=kT[mc][:, ki * 128:(ki + 1) * 128],
                                    rhs=qt[:, par, mc, trim:QCW],
                                    start=True,
                                    stop=True,
                                )
                            # exp(scale*s - ln16): the 1/16 keeps l within
                            # fp16 range for the broadcast matmul; it cancels
                            # in attn = (pav/16) * (16/l).
                            t0 = block_trim(qc, pair[0][1])
                            pT = ptl.tile([128, 2 * QCW], F16, name="pT", tag="pT")
                            nc.scalar.activation(pT[:, t0:w], pS[:, t0:w], AF.Exp,
                                                 scale=scale, bias=nbias)
                            for sj, (ki, pat) in enumerate(pair):
                                sl = pT[:, sj * QCW:(sj + 1) * QCW]
                                if pat is not None:
                                    kind, arg = pat
                                    if kind == "tri":
                                        # keep where (q - k) >= 0, else 0
                                        nc.gpsimd.affine_select(
                                            out=sl,
                                            in_=sl,
                                            pattern=[[1, QCW]],
                                            base=arg,
                                            channel_multiplier=-1,
                                            compare_op=mybir.AluOpType.is_ge,
                                            fill=0.0,
                                        )
                                    else:
                                        nc.gpsimd.tensor_mul(sl, sl, mts[arg])
                                nc.tensor.matmul(
                                    pav,
                                    lhsT=vS[:, ki, h],
                                    rhs=sl,
                                    start=(pi == 0 and sj == 0),
                                    stop=(pi + sj == len(row) - 1),
                                )
                        # normalize: 1/l on DVE (SBUF-fed fast reciprocal),
                        # broadcast across HD partitions via a DRAM bounce on
                        # the sync queue, then scale. (A gpsimd
                        # partition_broadcast would be one op, but it thrashes
                        # the DSP library against affine_select — 5us reload
                        # stalls.) For the last chunk the bounce latency is
                        # the kernel tail, so broadcast with a PE ones-matmul
                        # + DVE copy instead.
                        lsb = rip.tile([1, QCW], F32, name="lsb", tag="lsb")
                        nc.vector.tensor_scalar_mul(lsb, pav[HD:HD + 1, :], 1.0)
                        rinv = rip.tile([1, QCW], F32, name="rinv", tag="rinv")
                        nc.vector.reciprocal_approx_fast(out=rinv, in_=lsb)
                        if qc + 1 < NQC:
                            scr = drp.tile([QCW], F32, name="scr", tag="scr")
                            nc.sync.dma_start(out=scr, in_=rinv)
                            rbs = rip.tile([HD, QCW], F32, name="rbs", tag="rbs")
                            nc.sync.dma_start(
                                out=rbs,
                                in_=bass.AP(tensor=scr.tensor, offset=scr.offset,
                                            ap=[[0, HD]] + list(scr.ap)),
                            )
                        else:
                            rinv16 = rip.tile([1, QCW], F16, name="rinv16", tag="rinv16")
                            nc.vector.tensor_scalar_mul(rinv16, rinv, 1.0)
                            bcR = pwork.tile([128, QCW], F32, name="bcR", tag="pw")
                            nc.tensor.matmul(bcR, lhsT=onesW, rhs=rinv16,
                                             start=True, stop=True)
                            rbs = rip.tile([HD, QCW], F32, name="rbs", tag="rbs")
                            nc.vector.tensor_scalar_mul(rbs, bcR[0:HD, :], 1.0)
                        nc.vector.tensor_mul(
                            attnT[mc][par * HD:(par + 1) * HD, :],
                            pav[0:HD, :],
                            rbs,
                        )
                    if qc + 1 < NQC:
                        qkv_chunk(qc + 1)
                    for tv in range(TPC):
                        for nch in range(NPCH):
                            drips.append((qc, attnT, tv, nch))
                # tail drain: rotate stores across all three DMA queues so
                # the final writes don't serialize behind one another
                tail_qs = [nc.sync, nc.gpsimd, nc.scalar]
                ti = 0
                while drips:
                    proj_unit(store_q=tail_qs[ti % 3])
                    ti += 1
    nc.compile()
    return nc


def classify_mask(mask_bool, T):
    """Classify S^T blocks [k-tile 128, q-chunk 512] as skip / full / mixed.

    mask_bool: [T, T] bool, mask_bool[q, k] = attend(q -> k).
    Returns (blocks, patterns): blocks[qc] = list of (ki, pat_idx|None),
    patterns = np.ndarray [n_pat, 128, QCW] float32.
    """
    QCW = min(512, T)
    NQC = T // QCW
    NKT = T // 128
    maskT = mask_bool.T  # [k, q]
    patterns = []
    pat_index = {}
    blocks = []
    for qc in range(NQC):
        row = []
        for ki in range(NKT):
            blk = maskT[ki * 128:(ki + 1) * 128, qc * QCW:(qc + 1) * QCW]
            if not blk.any():
                continue
            if blk.all():
                row.append((ki, None))
                continue
            # tril-offset block? keep iff k <= q, i.e. p <= base + f
            base = qc * QCW - ki * 128
            p = np.arange(128)[:, None]
            f = np.arange(QCW)[None, :]
            if np.array_equal(blk, p <= base + f):
                row.append((ki, ("tri", base)))
                continue
            key = blk.tobytes()
            if key not in pat_index:
                pat_index[key] = len(patterns)
                patterns.append(blk.astype(np.float32))
            row.append((ki, ("pat", pat_index[key])))
        blocks.append(row)
    n_pat = len(patterns)
    if patterns:
        pats = np.stack(patterns)
    else:
        pats = np.zeros((1, 128, QCW), np.float32)
    return blocks, pats, n_pat


_prog_cache = {}


def _get_program(T, D, HL, mask_bool):
    key = (T, D, HL, mask_bool.tobytes())
    if key not in _prog_cache:
        blocks, pats, n_pat = classify_mask(mask_bool, T)
        nc = build_program(T, D, HL, n_pat, blocks)
        _prog_cache[key] = (nc, blocks, pats)
    return _prog_cache[key]


def kernel(x, W_qkv, b_qkv, W_proj, b_proj, mask):
    out, _ = run_attention(x, W_qkv, b_qkv, W_proj, b_proj, mask)
    return out


def run_attention(x, W_qkv, b_qkv, W_proj, b_proj, mask, trace=False):
    x = np.ascontiguousarray(np.asarray(x, dtype=np.float32))
    W_qkv = np.asarray(W_qkv, dtype=np.float32)
    b_qkv = np.asarray(b_qkv, dtype=np.float32)
    W_proj = np.asarray(W_proj, dtype=np.float32)
    b_proj = np.asarray(b_proj, dtype=np.float32)
    Bc, T, D = x.shape
    NH = NH_FULL
    HL = NH // 2  # heads per core (two head-groups)
    CL = HL * HD

    mask_bool = np.asarray(mask)[0, 0] != 0

    nc, blocks, pats = _get_program(T, D, HL, mask_bool)

    NDT = D // 128
    NCT = CL // 128
    QCW = min(512, T)
    NQC = T // QCW

    def tile_w(w, inner):
        # [D_rows, W_cols] -> [128(p), D_rows//128(n), *inner] partition-major
        return np.ascontiguousarray(
            w.reshape(w.shape[0] // 128, 128, *inner).transpose(
                1, 0, *range(2, 2 + len(inner)))
        ).astype(np.float16)

    def tile_w_ct(w):
        # [D_rows, NCT*128] -> [NCT, 128(p), NDT, 128] c-tile-major
        return np.ascontiguousarray(
            w.reshape(NDT, 128, NCT, 128).transpose(2, 1, 0, 3)
        ).astype(np.float16)

    in_maps = []
    n_cores = 2 * Bc
    for c in range(n_cores):
        b, g = c // 2, c % 2
        sl = slice(g * CL, (g + 1) * CL)
        xT = x[b].T  # [D, T]
        xhp = np.ascontiguousarray(
            xT.reshape(NDT, 128, NQC, QCW).transpose(2, 1, 0, 3)
        ).astype(np.float16)
        bq = b_qkv[0 * D:1 * D][sl]
        bk = b_qkv[1 * D:2 * D][sl]
        bv = b_qkv[2 * D:3 * D][sl]
        bp = b_proj if g == 0 else np.zeros_like(b_proj)
        bh = np.concatenate([
            bq.reshape(NCT, 128).T,           # [128, NCT] per-partition bqs
            bk.reshape(NCT, 128).T,           # [128, NCT]
            np.broadcast_to(bv, (128, CL)),   # [128, CL]
            np.broadcast_to(bp, (128, D)),    # [128, D]
        ], axis=1).astype(np.float32)
        in_maps.append({
            "xh": xhp,
            "wqh": tile_w_ct(W_qkv[:, 0 * D:1 * D][:, sl]),
            "wkh": tile_w_ct(W_qkv[:, 1 * D:2 * D][:, sl]),
            "wvh": tile_w(W_qkv[:, 2 * D:3 * D][:, sl], (CL,)),
            "wph": tile_w(W_proj[sl, :], (D,)),
            "bh": np.ascontiguousarray(bh),
            "mp": pats.astype(np.float16),
        })

    res = run_bass_kernel_spmd(nc, in_maps, list(range(n_cores)), trace=trace)
    out = np.empty((Bc, T, D), np.float32)
    for b in range(Bc):
        out[b] = res.results[2 * b]["y"] + res.results[2 * b + 1]["y"]
    return out, res


# revision 32
# speedup vs baseline: 1.0626x; 1.0046x over previous
"""Multi-head causal attention (B=4, T=2048, D=1024, 16 heads) on 8 TRN2 cores.

Sharding: core c -> batch b = c//2, head-group g = c%2 (8 of 16 heads).
Each core computes its batch's QKV for its heads, flash-style causal
attention with scores kept transposed (S^T[k, q]) so softmax sums come
free via a ones-column appended to V, then a partial output projection
y_part = attn_local @ W_proj[rows]. Host sums the two head-group partials
per batch.

Single fused pipeline per q-chunk (QKV for the chunk -> attention ->
projection) with all tile pools live at once, so the scheduler overlaps
next-chunk QKV matmuls into the ACT-paced attention stretches. x arrives
pre-transposed from the host (no device xbar transposes); weights load
per-c-tile across four queues so the first q-projection starts as soon
as ~384KB has landed instead of after the full 3.7MB weight set. y
stores at [128,512] granularity straight off each projection half.
Softmax normalization uses reciprocal_approx_fast plus a DRAM-bounce
row broadcast (ones-matmul for the last chunk). Projection drips are
deferred by one extra chunk (proj(0)->chunk2, proj(1,2)->chunk3) so the
final ACT-paced attention stretch has PE fill-in work.

Matmul operands are fp16; accumulation stays fp32 in PSUM.
"""

import math
from collections import deque
from contextlib import ExitStack

import numpy as np

import concourse.bacc as bacc
import concourse.bass as bass
import concourse.mybir as mybir
import concourse.tile as tile
from concourse.bass_utils import run_bass_kernel_spmd

AF = mybir.ActivationFunctionType
F32 = mybir.dt.float32
F16 = mybir.dt.float16

B_FULL = 4
T_FULL = 2048
D_FULL = 1024
NH_FULL = 16
HD = 64


def build_program(T, D, HL, n_pat, blocks):
    """Build the per-core SPMD program.

    T: sequence length, D: model dim, HL: local heads, n_pat: number of
    distinct mixed-mask pattern tiles, blocks: per q-chunk list of
    (k_tile_index, pattern_index_or_None) for active score blocks.
    """
    CL = HL * HD            # local channels (q, k, or v width)
    NDT = D // 128          # d-tiles (contraction tiles for qkv matmuls)
    NTT = T // 128          # t-tiles
    QCW = min(512, T)       # q-chunk width
    NQC = T // QCW
    TPC = QCW // 128        # t-tiles per q-chunk
    NCT = CL // 128         # c-tiles for q/k/attn storage
    PCH = min(512, D)       # proj output chunk
    NPCH = D // PCH
    scale = 1.0 / math.sqrt(HD)

    nc = bacc.Bacc("TRN2", target_bir_lowering=False, debug=False)
    # host-pre-tiled partition-major layouts so every load moves contiguous
    # multi-KB lines per partition (small-line APs run at ~1/4 DMA rate).
    # wq/wk are c-tile-major so the first projection's 256KB slab can land
    # ahead of the rest of the weight set.
    xh = nc.dram_tensor("xh", [NQC, 128, NDT, QCW], F16, kind="ExternalInput").ap()
    wqh = nc.dram_tensor("wqh", [NCT, 128, NDT, 128], F16, kind="ExternalInput").ap()
    wkh = nc.dram_tensor("wkh", [NCT, 128, NDT, 128], F16, kind="ExternalInput").ap()
    wvh = nc.dram_tensor("wvh", [128, NDT, CL], F16, kind="ExternalInput").ap()
    wph = nc.dram_tensor("wph", [128, NCT, D], F16, kind="ExternalInput").ap()
    # host-pre-broadcast bias pack: [bqs(NCT) | bks(NCT) | bvb(CL) | bpb(D)]
    BW = 2 * NCT + CL + D
    bh = nc.dram_tensor("bh", [128, BW], F32, kind="ExternalInput").ap()
    mp = nc.dram_tensor("mp", [max(n_pat, 1), 128, QCW], F16, kind="ExternalInput").ap()
    y = nc.dram_tensor("y", [T, D], F32, kind="ExternalOutput").ap()

    with tile.TileContext(nc) as tc, nc.allow_low_precision(
        reason="fp16 operands with fp32 PSUM accumulation; approx reciprocal"
    ):
        with ExitStack() as octx:
            persist = octx.enter_context(tc.tile_pool(name="persist", bufs=1))
            # K^T per c-tile, full sequence (grows as chunks complete)
            kT = [persist.tile([128, T], F16, name=f"kT{i}", tag=f"kT{i}") for i in range(NCT)]
            # small constants first: the HAM-warmup matmuls gate on onesW,
            # so it must clear the gpsimd queue quickly
            onesW = persist.tile([1, 128], F16, name="onesW", tag="onesW")
            nc.gpsimd.memset(onesW, 1.0)
            # -ln(16) bias column for the scaled exp
            nbias = persist.tile([128, 1], F32, name="nbias", tag="nbias")
            nc.gpsimd.memset(nbias, -math.log(16.0))
            # V in per-(t-tile, head) 128-wide slots: cols [0:HD) data, [HD]
            # ones (for the free softmax row-sum), rest junk. The junk lanes
            # only feed matmul out-partitions that are never read, so they
            # stay uninitialized on purpose (no startup memset wave).
            vS = persist.tile([128, NTT, HL, 128], F16, name="vS", tag="vS")

            # startup DMA plan, first-needed-first across four queues:
            #   sync:   bias qk slice, x chunk0 (d-tiles 0:2 then 2:8)
            #   scalar: wq per c-tile (first q matmul needs only ct 0)
            #   gpsimd: wk per c-tile (behind the small memsets), bpb, wps
            #   vector: wv, bvb
            bpack = persist.tile([128, BW], F32, name="bpack", tag="bpack")
            bqs = bpack[:, 0:NCT]
            bks = bpack[:, NCT:2 * NCT]
            bvb = bpack[:, 2 * NCT:2 * NCT + CL]
            bpb = bpack[:, 2 * NCT + CL:BW]
            nc.sync.dma_start(out=bpack[:, 0:2 * NCT], in_=bh[:, 0:2 * NCT])
            wqt = persist.tile([128, NCT, NDT, 128], F16, name="wqt", tag="wqt")
            wkt = persist.tile([128, NCT, NDT, 128], F16, name="wkt", tag="wkt")
            wvt = persist.tile([128, NDT, CL], F16, name="wvt", tag="wvt")
            # first DMA wave is ONLY x chunk0 + wq c-tile 0 (+ the bias
            # sliver): everything else is gated on x0's arrival below, so
            # the first matmul's inputs don't share HBM rings with 4.5MB of
            # weights that aren't needed for another 10us.
            nc.scalar.dma_start(out=wqt[:, 0], in_=wqh[0])
            gate = nc.dram_tensor("gate_scr", [4], F16, kind="Internal").ap()
            mts = [persist.tile([128, QCW], F16, name=f"mt{i}", tag=f"mt{i}") for i in range(n_pat)]

            # preload the exp table set during the initial DMA wait
            warmup = persist.tile([1, 16], F32, name="warm", tag="warm")
            nc.gpsimd.memset(warmup, 0.0)
            nc.scalar.activation(warmup, warmup, AF.Exp, scale=1.0)

            pools = ExitStack()
            with pools:
                xtp = pools.enter_context(tc.tile_pool(name="xtp", bufs=2))
                qtp = pools.enter_context(tc.tile_pool(name="qtp", bufs=2))
                attnp = pools.enter_context(tc.tile_pool(name="attnp", bufs=4))
                ptl = pools.enter_context(tc.tile_pool(name="ptl", bufs=3))
                rip = pools.enter_context(tc.tile_pool(name="rip", bufs=4))
                drp = pools.enter_context(tc.tile_pool(name="drp", bufs=4, space="DRAM"))
                ysb = pools.enter_context(tc.tile_pool(name="ysb", bufs=3))
                pss = pools.enter_context(tc.tile_pool(name="pss", bufs=2, space="PSUM"))
                psav = pools.enter_context(tc.tile_pool(name="psav", bufs=2, space="PSUM"))
                pwork = pools.enter_context(tc.tile_pool(name="pwork", bufs=2, space="PSUM"))

                # qT constant-zero halves for the ping-pong parity scheme:
                # parity e stores q in rows [0:HD) (rows [HD:128) stay 0),
                # parity o in rows [HD:128). The zero rows kill the other
                # head's channels in the 128-deep scores contraction. Only
                # chunk-0's buffer heads the DVE queue (must beat the first
                # bias-add there); everything not needed before ~30us goes
                # behind the gpsimd DMA issues or after the qkv0 emission.
                qbufs = [qtp.tile([128, 2, NCT, QCW], F16, name="qT", tag="qT")
                         for _ in range(2)]
                nc.vector.memset(qbufs[0][HD:128, 0], 0.0)
                nc.vector.memset(qbufs[0][0:HD, 1], 0.0)
                nc.gpsimd.memset(vS[:, :, :, HD:HD + 1], 1.0)
                # initialize the pT ring (trimmed exp leaves leading columns
                # stale on later passes; affine_select re-fills them)
                wu_pt = [ptl.tile([128, 2 * QCW], F16, name="pT", tag="pT") for _ in range(3)]
                for t in wu_pt:
                    nc.gpsimd.memset(t, 0.0)
                wps = persist.tile([128, NCT, D], F16, name="wps", tag="wps")

                # HAM warmup: keep the PE busy during the initial DMA wait so
                # the first real matmuls run at full clock. The first writes
                # fully initialize both pss pair buffers so the diagonal
                # trims can later leave stale (but initialized) PSUM columns.
                wu_ps = [pss.tile([128, 2 * QCW], F32, name="pS", tag="pS") for _ in range(2)]
                for i in range(2 * (2 * QCW // 128)):
                    dst = wu_ps[i % 2][:, (i // 2) * 128:(i // 2) * 128 + 128]
                    nc.tensor.matmul(dst, lhsT=onesW, rhs=onesW[:, 0:128],
                                     start=True, stop=True)
                wdum = pwork.tile([128, QCW], F32, name="wdum", tag="pw")
                for i in range(28):
                    nc.tensor.matmul(wdum[:, 0:128], lhsT=onesW, rhs=onesW[:, 0:128],
                                     start=(i == 0), stop=(i == 27))

                def qkv_chunk(ntc, xTc=None):
                    """QKV projections for q-chunk ntc: fills vS t-tiles,
                    kT columns, and the qT ping-pong buffer for this chunk."""
                    # one DMA per chunk: a single descriptor set spreads
                    # across all 16 rings, so 1MB lands in ~3us — faster
                    # than any split with its serial descriptor-gen cost
                    if xTc is None:
                        xTc = xtp.tile([128, NDT, QCW], F16, name="xTc", tag="xTc")
                        nc.sync.dma_start(out=xTc, in_=xh[ntc])
                    qt = qbufs[ntc % 2]

                    def qk_group(isq, mc):
                        wt = wqt if isq else wkt
                        pb = pwork.tile([128, QCW], F32, name="pb", tag="pw")
                        for dd in range(NDT):
                            nc.tensor.matmul(
                                pb,
                                lhsT=wt[:, mc, dd, :],
                                rhs=xTc[:, dd, :],
                                start=(dd == 0),
                                stop=(dd == NDT - 1),
                            )
                        if isq:
                            nc.vector.tensor_scalar_add(
                                qt[0:HD, 0, mc, :], pb[0:HD, :], bqs[0:HD, mc:mc + 1]
                            )
                            nc.vector.tensor_scalar_add(
                                qt[HD:128, 1, mc, :], pb[HD:128, :], bqs[HD:128, mc:mc + 1]
                            )
                        else:
                            nc.vector.tensor_scalar_add(
                                kT[mc][:, ntc * QCW:(ntc + 1) * QCW], pb, bks[:, mc:mc + 1]
                            )

                    def v_group(tv):
                        tt = ntc * TPC + tv
                        pv = pwork.tile([128, CL], F32, name="pv", tag="pw")
                        for dd in range(NDT):
                            nc.tensor.matmul(
                                pv,
                                lhsT=xTc[:, dd, tv * 128:(tv + 1) * 128],
                                rhs=wvt[:, dd, :],
                                start=(dd == 0),
                                stop=(dd == NDT - 1),
                            )
                        nc.vector.tensor_add(
                            vS[:, tt, :, 0:HD],
                            pv.rearrange("p (h d) -> p h d", h=HL),
                            bvb.rearrange("p (h d) -> p h d", h=HL),
                        )

                    # q/k first: their weights stream in c-tile order, so
                    # compute starts before the v/k weight sets finish
                    for mc in range(NCT):
                        qk_group(True, mc)
                        qk_group(False, mc)
                    for tv in range(TPC):
                        v_group(tv)

                # second DMA wave, gated on x chunk0's arrival: each queue
                # first writes a cell of xTc back to scratch DRAM, which
                # can't issue until the x0 load lands — keeping these 4.5MB
                # of weight transfers off the rings until then. Emitted
                # BEFORE qkv_chunk(0) so the weight writes stay ordered
                # ahead of their first readers.
                xTc0 = xtp.tile([128, NDT, QCW], F16, name="xTc", tag="xTc")
                nc.sync.dma_start(out=xTc0, in_=xh[0])
                nc.scalar.dma_start(out=gate[0:1], in_=xTc0[0:1, 0, 0:1])
                for mc in range(1, NCT):
                    nc.scalar.dma_start(out=wqt[:, mc], in_=wqh[mc])
                nc.scalar.dma_start(out=wvt, in_=wvh)
                nc.scalar.dma_start(
                    out=bpack[:, 2 * NCT:2 * NCT + CL], in_=bh[:, 2 * NCT:2 * NCT + CL]
                )
                for i in range(n_pat):
                    nc.scalar.dma_start(out=mts[i], in_=mp[i])
                nc.gpsimd.dma_start(out=gate[1:2], in_=xTc0[0:1, 0, 1:2])
                for mc in range(NCT):
                    nc.gpsimd.dma_start(out=wkt[:, mc], in_=wkh[mc])
                nc.gpsimd.dma_start(
                    out=bpack[:, 2 * NCT + CL:BW], in_=bh[:, 2 * NCT + CL:BW]
                )
                nc.gpsimd.dma_start(out=wps, in_=wph)
                qkv_chunk(0, xTc=xTc0)
                nc.vector.memset(qbufs[1][HD:128, 0], 0.0)
                nc.vector.memset(qbufs[1][0:HD, 1], 0.0)

                def block_trim(qc, pat):
                    """Fully-masked leading q-columns of a diagonal block
                    (skipped in matmuls and exp; affine_select re-zeroes
                    whatever stale values sit there)."""
                    if pat is not None and pat[0] == "tri" and pat[1] < 0:
                        return -pat[1]
                    return 0

                # projection drip queue: one (qc, attnT, tv, nch) unit per
                # [128, PCH] output block. All of chunks 0-2's units drain
                # inside chunk 3's head loops (the only ACT-paced stretch
                # without a next-chunk QKV reservoir), remainder at the tail.
                drips = deque()

                def proj_unit(store_q=None):
                    """Emit one (t-tile, out-chunk) projection unit of a
                    finished q-chunk; interleaved into later head loops so
                    its matmuls fill ACT-paced PE gaps. Stores its half-row
                    immediately to keep the kernel tail short."""
                    if not drips:
                        return
                    qc, attnT, tv, nch = drips.popleft()
                    py = pwork.tile([128, PCH], F32, name="py", tag="pw")
                    for cc in range(NCT):
                        nc.tensor.matmul(
                            py,
                            lhsT=attnT[cc][:, tv * 128:(tv + 1) * 128],
                            rhs=wps[:, cc, nch * PCH:(nch + 1) * PCH],
                            start=(cc == 0),
                            stop=(cc == NCT - 1),
                        )
                    yt = ysb.tile([128, PCH], F32, name="yt", tag="yt")
                    nc.vector.tensor_add(yt, py, bpb[:, nch * PCH:(nch + 1) * PCH])
                    tt = qc * TPC + tv
                    (store_q or nc.sync).dma_start(
                        out=y[tt * 128:(tt + 1) * 128, nch * PCH:(nch + 1) * PCH],
                        in_=yt,
                    )

                assert NQC == 4, "drip schedule is tuned for 4 q-chunks"
                drips_per_head = {0: 0, 1: 0, 2: 0, 3: 3}

                for qc in range(NQC):
                    row = blocks[qc]
                    assert row, f"q-chunk {qc} has no active k-tiles"
                    qt = qbufs[qc % 2]
                    attnT = [attnp.tile([128, QCW], F16, name=f"attnT{i}", tag=f"attnT{i}")
                             for i in range(NCT)]
                    npr = (len(row) + 1) // 2
                    dp = drips_per_head[qc]
                    # late-biased drip positions: the exp-paced PE deficit
                    # shows up at the head's trailing pairs / head boundary
                    drip_at = {2 * (npr - 1 - 2 * j) for j in range(dp)}
                    for h in range(HL):
                        mc, par = h // 2, h % 2
                        pav = psav.tile([128, QCW], F32, name="pav", tag="pav")
                        for pi in range(0, len(row), 2):
                            if pi in drip_at:
                                proj_unit()
                            pair = row[pi:pi + 2]
                            w = len(pair) * QCW
                            pS = pss.tile([128, 2 * QCW], F32, name="pS", tag="pS")
                            for sj, (ki, pat) in enumerate(pair):
                                trim = block_trim(qc, pat)
                                nc.tensor.matmul(
                                    pS[:, sj * QCW + trim:(sj + 1) * QCW],
                                    lhsT=kT[mc][:, ki * 128:(ki + 1) * 128],
                                    rhs=qt[:, par, mc, trim:QCW],
                                    start=True,
                                    stop=True,
                                )
                            # exp(scale*s - ln16): the 1/16 keeps l within
                            # fp16 range for the broadcast matmul; it cancels
                            # in attn = (pav/16) * (16/l).
                            t0 = block_trim(qc, pair[0][1])
                            pT = ptl.tile([128, 2 * QCW], F16, name="pT", tag="pT")
                            nc.scalar.activation(pT[:, t0:w], pS[:, t0:w], AF.Exp,
                                                 scale=scale, bias=nbias)
                            for sj, (ki, pat) in enumerate(pair):
                                sl = pT[:, sj * QCW:(sj + 1) * QCW]
                                if pat is not None:
                                    kind, arg = pat
                                    if kind == "tri":
                                        # keep where (q - k) >= 0, else 0
                                        nc.gpsimd.affine_select(
                                            out=sl,
                                            in_=sl,
                                            pattern=[[1, QCW]],
                                            base=arg,
                                            channel_multiplier=-1,
                                            compare_op=mybir.AluOpType.is_ge,
                                            fill=0.0,
                                        )
                                    else:
                                        nc.gpsimd.tensor_mul(sl, sl, mts[arg])
                                nc.tensor.matmul(
                                    pav,
                                    lhsT=vS[:, ki, h],
                                    rhs=sl,
                                    start=(pi == 0 and sj == 0),
                                    stop=(pi + sj == len(row) - 1),
                                )
                        # normalize: 1/l on DVE (SBUF-fed fast reciprocal),
                        # broadcast across HD partitions via a DRAM bounce on
                        # the sync queue, then scale. (A gpsimd
                        # partition_broadcast would be one op, but it thrashes
                        # the DSP library against affine_select — 5us reload
                        # stalls.) For the last chunk the bounce latency is
                        # the kernel tail, so broadcast with a PE ones-matmul
                        # + DVE copy instead.
                        lsb = rip.tile([1, QCW], F32, name="lsb", tag="lsb")
                        nc.vector.tensor_scalar_mul(lsb, pav[HD:HD + 1, :], 1.0)
                        rinv = rip.tile([1, QCW], F32, name="rinv", tag="rinv")
                        nc.vector.reciprocal_approx_fast(out=rinv, in_=lsb)
                        scr = drp.tile([QCW], F32, name="scr", tag="scr")
                        nc.sync.dma_start(out=scr, in_=rinv)
                        rbs = rip.tile([HD, QCW], F32, name="rbs", tag="rbs")
                        nc.sync.dma_start(
                            out=rbs,
                            in_=bass.AP(tensor=scr.tensor, offset=scr.offset,
                                        ap=[[0, HD]] + list(scr.ap)),
                        )
                        nc.vector.tensor_mul(
                            attnT[mc][par * HD:(par + 1) * HD, :],
                            pav[0:HD, :],
                            rbs,
                        )
                    if qc + 1 < NQC:
                        qkv_chunk(qc + 1)
                    for tv in range(TPC):
                        for nch in range(NPCH):
                            drips.append((qc, attnT, tv, nch))
                # tail drain: rotate stores across all three DMA queues so
                # the final writes don't serialize behind one another
                tail_qs = [nc.sync, nc.gpsimd, nc.scalar]
                ti = 0
                while drips:
                    proj_unit(store_q=tail_qs[ti % 3])
                    ti += 1
    nc.compile()
    return nc


def classify_mask(mask_bool, T):
    """Classify S^T blocks [k-tile 128, q-chunk 512] as skip / full / mixed.

    mask_bool: [T, T] bool, mask_bool[q, k] = attend(q -> k).
    Returns (blocks, patterns): blocks[qc] = list of (ki, pat_idx|None),
    patterns = np.ndarray [n_pat, 128, QCW] float32.
    """
    QCW = min(512, T)
    NQC = T // QCW
    NKT = T // 128
    maskT = mask_bool.T  # [k, q]
    patterns = []
    pat_index = {}
    blocks = []
    for qc in range(NQC):
        row = []
        for ki in range(NKT):
            blk = maskT[ki * 128:(ki + 1) * 128, qc * QCW:(qc + 1) * QCW]
            if not blk.any():
                continue
            if blk.all():
                row.append((ki, None))
                continue
            # tril-offset block? keep iff k <= q, i.e. p <= base + f
            base = qc * QCW - ki * 128
            p = np.arange(128)[:, None]
            f = np.arange(QCW)[None, :]
            if np.array_equal(blk, p <= base + f):
                row.append((ki, ("tri", base)))
                continue
            key = blk.tobytes()
            if key not in pat_index:
                pat_index[key] = len(patterns)
                patterns.append(blk.astype(np.float32))
            row.append((ki, ("pat", pat_index[key])))
        blocks.append(row)
    n_pat = len(patterns)
    if patterns:
        pats = np.stack(patterns)
    else:
        pats = np.zeros((1, 128, QCW), np.float32)
    return blocks, pats, n_pat


_prog_cache = {}


def _get_program(T, D, HL, mask_bool):
    key = (T, D, HL, mask_bool.tobytes())
    if key not in _prog_cache:
        blocks, pats, n_pat = classify_mask(mask_bool, T)
        nc = build_program(T, D, HL, n_pat, blocks)
        _prog_cache[key] = (nc, blocks, pats)
    return _prog_cache[key]


def kernel(x, W_qkv, b_qkv, W_proj, b_proj, mask):
    out, _ = run_attention(x, W_qkv, b_qkv, W_proj, b_proj, mask)
    return out


def run_attention(x, W_qkv, b_qkv, W_proj, b_proj, mask, trace=False):
    x = np.ascontiguousarray(np.asarray(x, dtype=np.float32))
    W_qkv = np.asarray(W_qkv, dtype=np.float32)
    b_qkv = np.asarray(b_qkv, dtype=np.float32)
    W_proj = np.asarray(W_proj, dtype=np.float32)
    b_proj = np.asarray(b_proj, dtype=np.float32)
    Bc, T, D = x.shape
    NH = NH_FULL
    HL = NH // 2  # heads per core (two head-groups)
    CL = HL * HD

    mask_bool = np.asarray(mask)[0, 0] != 0

    nc, blocks, pats = _get_program(T, D, HL, mask_bool)

    NDT = D // 128
    NCT = CL // 128
    QCW = min(512, T)
    NQC = T // QCW

    def tile_w(w, inner):
        # [D_rows, W_cols] -> [128(p), D_rows//128(n), *inner] partition-major
        return np.ascontiguousarray(
            w.reshape(w.shape[0] // 128, 128, *inner).transpose(
                1, 0, *range(2, 2 + len(inner)))
        ).astype(np.float16)

    def tile_w_ct(w):
        # [D_rows, NCT*128] -> [NCT, 128(p), NDT, 128] c-tile-major
        return np.ascontiguousarray(
            w.reshape(NDT, 128, NCT, 128).transpose(2, 1, 0, 3)
        ).astype(np.float16)

    in_maps = []
    n_cores = 2 * Bc
    for c in range(n_cores):
        b, g = c // 2, c % 2
        sl = slice(g * CL, (g + 1) * CL)
        xT = x[b].T  # [D, T]
        xhp = np.ascontiguousarray(
            xT.reshape(NDT, 128, NQC, QCW).transpose(2, 1, 0, 3)
        ).astype(np.float16)
        bq = b_qkv[0 * D:1 * D][sl]
        bk = b_qkv[1 * D:2 * D][sl]
        bv = b_qkv[2 * D:3 * D][sl]
        bp = b_proj if g == 0 else np.zeros_like(b_proj)
        bh = np.concatenate([
            bq.reshape(NCT, 128).T,           # [128, NCT] per-partition bqs
            bk.reshape(NCT, 128).T,           # [128, NCT]
            np.broadcast_to(bv, (128, CL)),   # [128, CL]
            np.broadcast_to(bp, (128, D)),    # [128, D]
        ], axis=1).astype(np.float32)
        in_maps.append({
            "xh": xhp,
            "wqh": tile_w_ct(W_qkv[:, 0 * D:1 * D][:, sl]),
            "wkh": tile_w_ct(W_qkv[:, 1 * D:2 * D][:, sl]),
            "wvh": tile_w(W_qkv[:, 2 * D:3 * D][:, sl], (CL,)),
            "wph": tile_w(W_proj[sl, :], (D,)),
            "bh": np.ascontiguousarray(bh),
            "mp": pats.astype(np.float16),
        })

    res = run_bass_kernel_spmd(nc, in_maps, list(range(n_cores)), trace=trace)
    out = np.empty((Bc, T, D), np.float32)
    for b in range(Bc):
        out[b] = res.results[2 * b]["y"] + res.results[2 * b + 1]["y"]
    return out, res
